# revision 55
# baseline (speedup 1.0000x reference)
"""Trainium2 Bass kernel for EHRCPCC loss (Pearson corr between condensed
pairwise L2 distances and a label-disagreement indicator over all B*(B-1)/2
upper-triangle pairs).

Strategy (8 NeuronCores, data-parallel over 512-row blocks of the gram
matrix):
  * Pearson needs only the moments Sx, Sxx, Sxy, Sy, N over the P pairs.
      - Sy = n0*n1, Syy = Sy (binary labels) -- exact on host.
      - Sxx = sum_{i<j} d2_ij = B*sum(sq) - ||sum_i x_i||^2 -- exact f64 host.
      - Sx, Sxy need the actual sqrt(d2) values -> computed on device.
  * d(i,i) = 0 and y(i,i) = 0, so sum over i<j == (sum over full matrix)/2;
    no triangle masking needed on device.
  * Rows/cols are permuted so labels are sorted. Then
    Sxy = sum_m [ C[m] + l_m * (A[m] - 2*C[m]) ] / 2, where A[m] is the full
    row sum of x and C[m] is the row sum over columns with label 1 -- a single
    contiguous column range. Per-tile row sums come free via the activation
    accum_out; only the 1024-col pair containing the label boundary needs one
    extra partial reduce.
  * x = sqrt(sq_i + sq_j - 2*g_ij + EPS): the +EPS keeps the (excluded)
    diagonal, where fp rounding can make d2 slightly negative, out of NaN
    territory without a clamp op. A constant shift cancels in Pearson
    centering; the diagonal's known sqrt(EPS) contribution is subtracted on
    the host.
  * Matmul inputs are bf16 (1 PE cycle/row; halves the replicated-X DMA);
    all host-side moments are computed from the SAME bf16-rounded values so
    device and host stay consistent. The stationary operand is pre-scaled by
    -2 (exact in bf16) so PSUM holds -2*gram directly.
  * Per [128,1024] column-pair: 4 matmuls, 1 DVE tensor_add (+ sq_j
    broadcast), 1 ACT Sqrt(+ per-partition sq_i bias) with accum_out row sum.
"""

import ml_dtypes
import numpy as np

import concourse.bass as bass
import concourse.mybir as mybir
from concourse import bacc
from concourse.bass_utils import run_bass_kernel_spmd
from concourse.tile import TileContext

B = 4096
D = 256
NCORES = 8
BLK = B // NCORES          # 512 rows per core
NT = BLK // 128            # 4 row tiles of 128 partitions
NP = B // 1024             # 4 column pairs of 1024
EPS = 1e-2

_PROGRAM_CACHE: dict = {}
LAST_RESULTS = None  # BassKernelResults of the most recent run (for profiling)


def _build_program(pb: int, poff: int):
    """Build the per-core Bass program. pb/poff locate the sorted-label
    boundary: boundary 1024-col pair index and offset within it (poff==0
    means the boundary is pair-aligned and no partial reduce is needed)."""
    f32 = mybir.dt.float32
    bf16 = mybir.dt.bfloat16
    nc = bacc.Bacc(None, target_bir_lowering=False, num_swdge_queues=2)

    xt_d = nc.dram_tensor("xt", [2 * 128, B], bf16, kind="ExternalInput")
    lt_d = nc.dram_tensor("lt", [128, 2 * BLK], bf16, kind="ExternalInput")
    sq_d = nc.dram_tensor("sq", [1, B], f32, kind="ExternalInput")
    sqr_d = nc.dram_tensor("sqr", [128, NT], f32, kind="ExternalInput")
    out_d = nc.dram_tensor("out", [128, 16 + NT], f32, kind="ExternalOutput")

    with TileContext(nc) as tc:
        with (
            tc.tile_pool(name="big", bufs=1) as big,
            tc.tile_pool(name="vp", bufs=4) as vp,
            tc.tile_pool(name="xp", bufs=4) as xp,
            tc.tile_pool(name="psum", bufs=4, space="PSUM") as pp,
        ):
            xt0 = big.tile([128, B], bf16, tag="xt0")
            xt1 = big.tile([128, B], bf16, tag="xt1")
            lts = big.tile([128, 2 * BLK], bf16, tag="lts")
            sqb = big.tile([128, B], f32, tag="sqb")
            sqr_s = big.tile([128, NT], f32, tag="sqr")
            racc = big.tile([128, 16], f32, tag="racc")
            rpart = big.tile([128, NT], f32, tag="rpart")

            # Spread input loads across independent DMA queues (one HWDGE
            # ring per issuing engine) so they run in parallel.
            # lt packed [128, 1024]: chunk k of the (-2x)-scaled stationary
            # operand lives at columns [k*512, (k+1)*512).
            nc.gpsimd.dma_start(out=lts, in_=lt_d[:, :])
            nc.gpsimd.dma_start(out=sqr_s, in_=sqr_d[:, :])
            # Broadcast sq [1,B] across all 128 partitions via a
            # partition-step-0 DRAM read (gpsimd SWDGE queue).
            sq_ap = sq_d[:, :]
            for h in range(4):
                hw = B // 4
                sq_bcast = bass.AP(
                    tensor=sq_ap.tensor,
                    offset=sq_ap.offset + h * hw,
                    ap=[[0, 128], [1, hw]],
                )
                nc.gpsimd.dma_start(out=sqb[:, h * hw:(h + 1) * hw], in_=sq_bcast)
            # Replicated X^T: one >=1 MiB transfer per K-chunk (full-rate
            # DMA), each on its own HWDGE queue.
            nc.sync.dma_start(out=xt0, in_=xt_d[0:128, :])
            nc.scalar.dma_start(out=xt1, in_=xt_d[128:256, :])

            if poff == 0:
                nc.vector.memset(rpart, 0.0)

            for t in range(NT):
                l0 = lts[:, t * 128:(t + 1) * 128]
                l1 = lts[:, 512 + t * 128:512 + (t + 1) * 128]
                for p in range(NP):
                    ps = pp.tile([128, 1024], f32, tag="ps")
                    for jh in range(2):
                        js = slice(p * 1024 + jh * 512, p * 1024 + (jh + 1) * 512)
                        ph = slice(jh * 512, (jh + 1) * 512)
                        nc.tensor.matmul(
                            ps[:, ph], l0, xt0[:, js], start=True, stop=False
                        )
                        nc.tensor.matmul(
                            ps[:, ph], l1, xt1[:, js], start=False, stop=True
                        )
                    pcols = slice(p * 1024, (p + 1) * 1024)
                    v = vp.tile([128, 1024], f32, tag="v")
                    # lt is pre-scaled by -2 on the host, so ps = -2*gram and
                    # v = sq_j - 2*gram.
                    nc.vector.tensor_add(v, ps, sqb[:, pcols])
                    x = xp.tile([128, 1024], f32, tag="x")
                    col = t * NP + p
                    nc.scalar.activation(
                        x, v, mybir.ActivationFunctionType.Sqrt,
                        bias=sqr_s[:, t:t + 1], scale=1.0,
                        accum_out=racc[:, col:col + 1],
                    )
                    if poff and p == pb:
                        nc.vector.reduce_sum(
                            out=rpart[:, t:t + 1], in_=x[:, poff:1024],
                            axis=mybir.AxisListType.X,
                        )

            nc.sync.dma_start(out=out_d[:, 0:16], in_=racc)
            nc.sync.dma_start(out=out_d[:, 16:16 + NT], in_=rpart)

    nc.finalize()
    return nc


def kernel(representations: np.ndarray, labels: np.ndarray) -> np.ndarray:
    X = np.ascontiguousarray(representations, dtype=np.float32)
    lab = np.asarray(labels).astype(np.int64)
    assert X.shape == (B, D)

    n0 = int((lab == 0).sum())
    n1 = B - n0
    Pn = B * (B - 1) // 2

    # Host-exact moments for the binary label vector.
    Sy = float(n0) * float(n1)
    vy = Sy - Sy * Sy / Pn
    if vy <= 0.0:
        # Zero label variance -> corr is NaN -> reference returns 1.0.
        return np.asarray(1.0, dtype=np.float32)

    # Sort points by label so label-1 columns are one contiguous range.
    perm = np.argsort(lab, kind="stable")
    Xs = X[perm]

    # The device matmul consumes bf16: all host-side moments must be computed
    # from the SAME rounded values so the (excluded) diagonal d2 stays ~0 and
    # Sxx is consistent with the device's sqrt(d2) values.
    Xb = Xs.astype(ml_dtypes.bfloat16)
    X64 = Xb.astype(np.float64)
    sq64 = np.einsum("ij,ij->i", X64, X64)
    s64 = X64.sum(axis=0)
    Sxx = B * sq64.sum() - s64 @ s64  # == sum_{i<j} ||x_i - x_j||^2, exact

    pb, poff = divmod(n0, 1024)

    key = (pb, poff)
    if key not in _PROGRAM_CACHE:
        _PROGRAM_CACHE[key] = _build_program(pb, poff)
    nc = _PROGRAM_CACHE[key]

    XT = np.ascontiguousarray(Xb.T)                     # [256, 4096] bf16
    sq32 = sq64.astype(np.float32)
    sq_in = np.ascontiguousarray(sq32.reshape(1, B))
    in_maps = []
    for c in range(NCORES):
        rs = slice(c * BLK, (c + 1) * BLK)
        # Stationary operand pre-scaled by -2 (exact in bf16), packed as
        # [128, 2*BLK]: K-chunk k at columns [k*BLK, (k+1)*BLK).
        ltc = XT[:, rs] * ml_dtypes.bfloat16(-2.0)      # [256, 512]
        lt = np.ascontiguousarray(
            np.concatenate([ltc[0:128, :], ltc[128:256, :]], axis=1)
        )                                               # [128, 1024]
        sqr = np.ascontiguousarray(
            sq32[rs].reshape(NT, 128).T + np.float32(EPS)
        )                                               # [128, 4]
        in_maps.append({"xt": XT, "lt": lt, "sq": sq_in, "sqr": sqr})

    res = run_bass_kernel_spmd(nc, in_maps, core_ids=list(range(NCORES)))
    global LAST_RESULTS
    LAST_RESULTS = res

    # Combine per-core row sums in f64.
    p_hi = pb + (1 if poff else 0)
    SxF = 0.0
    SxyF = 0.0
    for c in range(NCORES):
        out = res.results[c]["out"].astype(np.float64)  # [128, 20]
        racc = out[:, :16].reshape(128, NT, NP)         # [m, t, p]
        rpart = out[:, 16:16 + NT]                      # [m, t]
        A = racc.sum(axis=2)                            # [m, t] full row sums
        C = racc[:, :, p_hi:].sum(axis=2)
        if poff:
            C = C + rpart
        m_idx = np.arange(128).reshape(128, 1)
        t_idx = np.arange(NT).reshape(1, NT)
        grow = c * BLK + t_idx * 128 + m_idx            # global sorted row id
        lmask = (grow >= n0).astype(np.float64)
        SxF += A.sum()
        SxyF += (C + lmask * (A - 2.0 * C)).sum()

    SxF -= B * np.sqrt(EPS)  # remove the diagonal's sqrt(EPS) contribution
    Sx = SxF / 2.0
    Sxy = SxyF / 2.0

    cov = Sxy - Sx * Sy / Pn
    vx = Sxx - Sx * Sx / Pn
    corr = cov / np.sqrt(vx * vy)
    loss = 1.0 - corr
    if not np.isfinite(loss):
        loss = 1.0
    return np.asarray(loss, dtype=np.float32)


# revision 56
# speedup vs baseline: 1.1065x; 1.1065x over previous
"""Trainium2 Bass kernel for EHRCPCC loss (Pearson corr between condensed
pairwise L2 distances and a label-disagreement indicator over all B*(B-1)/2
upper-triangle pairs).

Strategy (8 NeuronCores, data-parallel over 512-row blocks of the gram
matrix):
  * Pearson needs only the moments Sx, Sxx, Sxy, Sy, N over the P pairs.
      - Sy = n0*n1, Syy = Sy (binary labels) -- exact on host.
      - Sxx = sum_{i<j} d2_ij = B*sum(sq) - ||sum_i x_i||^2 -- exact f64 host.
      - Sx, Sxy need the actual sqrt(d2) values -> computed on device.
  * d(i,i) = 0 and y(i,i) = 0, so sum over i<j == (sum over full matrix)/2;
    no triangle masking needed on device.
  * Rows/cols are permuted so labels are sorted. Then
    Sxy = sum_m [ C[m] + l_m * (A[m] - 2*C[m]) ] / 2, where A[m] is the full
    row sum of x and C[m] is the row sum over columns with label 1 -- a single
    contiguous column range. Per-tile row sums come free via the activation
    accum_out; only the 1024-col pair containing the label boundary needs one
    extra partial reduce.
  * x = sqrt(sq_i + sq_j - 2*g_ij + EPS): the +EPS keeps the (excluded)
    diagonal, where fp rounding can make d2 slightly negative, out of NaN
    territory without a clamp op. A constant shift cancels in Pearson
    centering; the diagonal's known sqrt(EPS) contribution is subtracted on
    the host.
  * Matmul inputs are bf16 (1 PE cycle/row; halves the replicated-X DMA);
    all host-side moments are computed from the SAME bf16-rounded values so
    device and host stay consistent. The stationary operand is pre-scaled by
    -2 (exact in bf16) so PSUM holds -2*gram directly.
  * Per [128,1024] column-pair: 4 matmuls, 1 DVE tensor_add (+ sq_j
    broadcast), 1 ACT Sqrt(+ per-partition sq_i bias) with accum_out row sum.
"""

import ml_dtypes
import numpy as np

import concourse.bass as bass
import concourse.mybir as mybir
from concourse import bacc
from concourse.bass_utils import run_bass_kernel_spmd
from concourse.tile import TileContext

B = 4096
D = 256
NCORES = 8
BLK = B // NCORES          # 512 rows per core
NT = BLK // 128            # 4 row tiles of 128 partitions
NP = B // 1024             # 4 column pairs of 1024
EPS = 1e-2

_PROGRAM_CACHE: dict = {}
LAST_RESULTS = None  # BassKernelResults of the most recent run (for profiling)


def _build_program(pb: int, poff: int):
    """Build the per-core Bass program. pb/poff locate the sorted-label
    boundary: boundary 1024-col pair index and offset within it (poff==0
    means the boundary is pair-aligned and no partial reduce is needed)."""
    f32 = mybir.dt.float32
    bf16 = mybir.dt.bfloat16
    fp8 = mybir.dt.float8e4
    nc = bacc.Bacc(None, target_bir_lowering=False, num_swdge_queues=2)

    xt_d = nc.dram_tensor("xt", [2 * 128, B], fp8, kind="ExternalInput")
    lt_d = nc.dram_tensor("lt", [128, 2 * BLK], fp8, kind="ExternalInput")
    sq_d = nc.dram_tensor("sq", [1, B], f32, kind="ExternalInput")
    sqr_d = nc.dram_tensor("sqr", [128, NT], f32, kind="ExternalInput")
    out_d = nc.dram_tensor("out", [128, 16 + NT], f32, kind="ExternalOutput")

    with TileContext(nc) as tc:
        with (
            tc.tile_pool(name="big", bufs=1) as big,
            tc.tile_pool(name="vp", bufs=4) as vp,
            tc.tile_pool(name="xp", bufs=4) as xp,
            tc.tile_pool(name="psum", bufs=4, space="PSUM") as pp,
        ):
            xt0 = big.tile([128, B], fp8, tag="xt0")
            xt1 = big.tile([128, B], fp8, tag="xt1")
            lts = big.tile([128, 2 * BLK], fp8, tag="lts")
            sqb = big.tile([128, B], f32, tag="sqb")
            sqr_s = big.tile([128, NT], f32, tag="sqr")
            racc = big.tile([128, 16], f32, tag="racc")
            rpart = big.tile([128, NT], f32, tag="rpart")

            # Spread input loads across independent DMA queues (one HWDGE
            # ring per issuing engine) so they run in parallel.
            # lt packed [128, 1024]: chunk k of the (-2x)-scaled stationary
            # operand lives at columns [k*512, (k+1)*512).
            nc.gpsimd.dma_start(out=lts, in_=lt_d[:, :])
            nc.gpsimd.dma_start(out=sqr_s, in_=sqr_d[:, :])
            # Broadcast sq [1,B] across all 128 partitions via a
            # partition-step-0 DRAM read (gpsimd SWDGE queue).
            sq_ap = sq_d[:, :]
            for h in range(4):
                hw = B // 4
                sq_bcast = bass.AP(
                    tensor=sq_ap.tensor,
                    offset=sq_ap.offset + h * hw,
                    ap=[[0, 128], [1, hw]],
                )
                nc.gpsimd.dma_start(out=sqb[:, h * hw:(h + 1) * hw], in_=sq_bcast)
            # Replicated X^T: one >=1 MiB transfer per K-chunk (full-rate
            # DMA), each on its own HWDGE queue.
            nc.sync.dma_start(out=xt0, in_=xt_d[0:128, :])
            nc.scalar.dma_start(out=xt1, in_=xt_d[128:256, :])

            if poff == 0:
                nc.vector.memset(rpart, 0.0)

            for t in range(NT):
                l0 = lts[:, t * 128:(t + 1) * 128]
                l1 = lts[:, 512 + t * 128:512 + (t + 1) * 128]
                for p in range(NP):
                    ps = pp.tile([128, 1024], f32, tag="ps")
                    for jh in range(2):
                        js = slice(p * 1024 + jh * 512, p * 1024 + (jh + 1) * 512)
                        ph = slice(jh * 512, (jh + 1) * 512)
                        nc.tensor.matmul(
                            ps[:, ph], l0, xt0[:, js], start=True, stop=False
                        )
                        nc.tensor.matmul(
                            ps[:, ph], l1, xt1[:, js], start=False, stop=True
                        )
                    pcols = slice(p * 1024, (p + 1) * 1024)
                    v = vp.tile([128, 1024], f32, tag="v")
                    # lt is pre-scaled by -2 on the host, so ps = -2*gram and
                    # v = sq_j - 2*gram.
                    nc.vector.tensor_add(v, ps, sqb[:, pcols])
                    x = xp.tile([128, 1024], f32, tag="x")
                    col = t * NP + p
                    nc.scalar.activation(
                        x, v, mybir.ActivationFunctionType.Sqrt,
                        bias=sqr_s[:, t:t + 1], scale=1.0,
                        accum_out=racc[:, col:col + 1],
                    )
                    if poff and p == pb:
                        nc.vector.reduce_sum(
                            out=rpart[:, t:t + 1], in_=x[:, poff:1024],
                            axis=mybir.AxisListType.X,
                        )

            nc.sync.dma_start(out=out_d[:, 0:16], in_=racc)
            nc.sync.dma_start(out=out_d[:, 16:16 + NT], in_=rpart)

    nc.finalize()
    return nc


def kernel(representations: np.ndarray, labels: np.ndarray) -> np.ndarray:
    X = np.ascontiguousarray(representations, dtype=np.float32)
    lab = np.asarray(labels).astype(np.int64)
    assert X.shape == (B, D)

    n0 = int((lab == 0).sum())
    n1 = B - n0
    Pn = B * (B - 1) // 2

    # Host-exact moments for the binary label vector.
    Sy = float(n0) * float(n1)
    vy = Sy - Sy * Sy / Pn
    if vy <= 0.0:
        # Zero label variance -> corr is NaN -> reference returns 1.0.
        return np.asarray(1.0, dtype=np.float32)

    # Sort points by label so label-1 columns are one contiguous range.
    perm = np.argsort(lab, kind="stable")
    Xs = X[perm]

    # The device matmul consumes bf16: all host-side moments must be computed
    # from the SAME rounded values so the (excluded) diagonal d2 stays ~0 and
    # Sxx is consistent with the device's sqrt(d2) values.
    Xb = Xs.astype(ml_dtypes.float8_e4m3)
    X64 = Xb.astype(np.float64)
    sq64 = np.einsum("ij,ij->i", X64, X64)
    s64 = X64.sum(axis=0)
    Sxx = B * sq64.sum() - s64 @ s64  # == sum_{i<j} ||x_i - x_j||^2, exact

    pb, poff = divmod(n0, 1024)

    key = (pb, poff)
    if key not in _PROGRAM_CACHE:
        _PROGRAM_CACHE[key] = _build_program(pb, poff)
    nc = _PROGRAM_CACHE[key]

    XT = np.ascontiguousarray(Xb.T)                     # [256, 4096] bf16
    sq32 = sq64.astype(np.float32)
    sq_in = np.ascontiguousarray(sq32.reshape(1, B))
    in_maps = []
    for c in range(NCORES):
        rs = slice(c * BLK, (c + 1) * BLK)
        # Stationary operand pre-scaled by -2 (exact in bf16), packed as
        # [128, 2*BLK]: K-chunk k at columns [k*BLK, (k+1)*BLK).
        ltc = (XT[:, rs].astype(np.float32) * np.float32(-2.0)).astype(
            ml_dtypes.float8_e4m3
        )                                               # [256, 512]
        lt = np.ascontiguousarray(
            np.concatenate([ltc[0:128, :], ltc[128:256, :]], axis=1)
        )                                               # [128, 1024]
        sqr = np.ascontiguousarray(
            sq32[rs].reshape(NT, 128).T + np.float32(EPS)
        )                                               # [128, 4]
        in_maps.append({"xt": XT, "lt": lt, "sq": sq_in, "sqr": sqr})

    res = run_bass_kernel_spmd(nc, in_maps, core_ids=list(range(NCORES)))
    global LAST_RESULTS
    LAST_RESULTS = res

    # Combine per-core row sums in f64.
    p_hi = pb + (1 if poff else 0)
    SxF = 0.0
    SxyF = 0.0
    for c in range(NCORES):
        out = res.results[c]["out"].astype(np.float64)  # [128, 20]
        racc = out[:, :16].reshape(128, NT, NP)         # [m, t, p]
        rpart = out[:, 16:16 + NT]                      # [m, t]
        A = racc.sum(axis=2)                            # [m, t] full row sums
        C = racc[:, :, p_hi:].sum(axis=2)
        if poff:
            C = C + rpart
        m_idx = np.arange(128).reshape(128, 1)
        t_idx = np.arange(NT).reshape(1, NT)
        grow = c * BLK + t_idx * 128 + m_idx            # global sorted row id
        lmask = (grow >= n0).astype(np.float64)
        SxF += A.sum()
        SxyF += (C + lmask * (A - 2.0 * C)).sum()

    SxF -= B * np.sqrt(EPS)  # remove the diagonal's sqrt(EPS) contribution
    Sx = SxF / 2.0
    Sxy = SxyF / 2.0

    cov = Sxy - Sx * Sy / Pn
    vx = Sxx - Sx * Sx / Pn
    corr = cov / np.sqrt(vx * vy)
    loss = 1.0 - corr
    if not np.isfinite(loss):
        loss = 1.0
    return np.asarray(loss, dtype=np.float32)
